# revision 6
# baseline (speedup 1.0000x reference)
"""MoE layer (top-2 of 8 experts, SwiGLU) on 8 Trainium2 NeuronCores.

Expert-parallel (per the sharding hint): the host computes the router
(gate logits -> top-2 -> softmax) in fp32, gathers each expert's tokens
(the "all-to-all dispatch"), each core runs a dense SwiGLU MLP over one
expert's tokens in bf16 (fp32 PSUM accumulation), and the host applies
the combine weights and scatter-adds back to token order.

Kernel structure (per core, C = max expert token count padded to 32):
  - stage-1 (h = silu(x@wg.T) * (x@w1.T)) in 512-token blocks plus one
    tail block; w1 resident in SBUF, wg streamed; weights stationary,
    tokens moving (N=512 streams keep LDWEIGHTS hidden).
  - stage-2 (y = h@w2.T) streams w2 two 128x512 chunks per DMA; the
    tail block's stage-2 is merged into the last full block's pass
    (5 psum tiles per w2 chunk) so no pass is w2-DMA-paced and w2 is
    streamed once less per forward.
  - y returned in bf16 (host upcasts; quantization well inside budget).
PSUM: psg 2 + ps1t 1 + psy 5 = 8 banks exactly.
"""

import numpy as np
import ml_dtypes

import concourse.bass as bass
import concourse.mybir as mybir
import concourse.tile as tile
from concourse.bass_utils import run_bass_kernel_spmd

# ---------------------------------------------------------------------------
# Workaround for this walrus build: TPB instructions have a single hardware
# wait slot; split k-wait instructions into k-1 single-wait NOPs + the
# original (program-order semantics identical).
# ---------------------------------------------------------------------------

_ws_counter = [0]


def _split_multi_waits(nc: bass.Bass) -> int:
    n_split = 0
    for f in nc.m.functions:
        for bb in f.blocks:
            new_insts = []
            for inst in bb.instructions:
                si = inst.sync_info
                if si is not None and si.on_wait and len(si.on_wait) > 1:
                    waits = list(si.on_wait)
                    for w in waits[:-1]:
                        _ws_counter[0] += 1
                        n_split += 1
                        new_insts.append(
                            mybir.InstNoOp(
                                name=f"waitsplit-{_ws_counter[0]}",
                                opcode="NoOp",
                                engine=inst.engine,
                                sync_info=mybir.SyncInfo(
                                    on_wait=[w], on_update=[]
                                ),
                                bass_nofuse=True,
                                text_hint="waitsplit",
                            )
                        )
                    si.on_wait = [waits[-1]]
                new_insts.append(inst)
            bb.instructions[:] = new_insts
    return n_split

# ---------------------------------------------------------------------------

D = 1024
DFF = 4096
N_EXPERTS = 8
TOP_K = 2
N_CORES = 8
TB = 512          # full token block
WG_BUFS = 4
XT_BUFS = 3
W2_BUFS = 8       # bufs of 2-chunk w2 tiles
H_BUFS = 36
PSG_BUFS = 2
PS1_BUFS = 1
PSY_BUFS = 5
KD = D // 128     # 8 contraction tiles over d
NF = DFF // 128   # 32 tiles over d_ff

BF16 = mybir.dt.bfloat16
F32 = mybir.dt.float32
NP_BF16 = ml_dtypes.bfloat16

_NC_CACHE: dict[int, bass.Bass] = {}


def _build_kernel(C: int, repeat: int = 1) -> bass.Bass:
    """Dense SwiGLU MLP over C tokens (C a multiple of 32).

    Blocks: one tail of C%512 (if any) first, then full 512 blocks. The
    tail's stage-2 is folded into the first full block's stage-2 pass.

    repeat>1 wraps the computation in a hardware For_i loop for
    wall-clock slope calibration (resident w1 loads once, outside)."""
    assert C % 32 == 0 and C >= 128
    tail = C % TB
    n_full = C // TB
    # Tail last: during the earlier blocks' stage-2 passes the wg stream
    # runs ahead, so the tail's fast-consuming stage-1 isn't DMA-paced.
    blocks = [TB] * n_full + ([tail] if tail else [])
    # stage-2 groups: indices of blocks whose stage-2 runs as one pass;
    # the tail's stage-2 is merged with the last full block's when the
    # combined m-tile count fits the PSY_BUFS psum banks.
    groups = [[i] for i in range(len(blocks))]
    if tail and n_full and TB // 128 + (tail + 127) // 128 <= PSY_BUFS:
        groups = groups[:-2] + [[len(blocks) - 2, len(blocks) - 1]]

    nc = bass.Bass()
    xt = nc.dram_tensor("xt", [128, KD, C], BF16, kind="ExternalInput")
    w1t = nc.dram_tensor("w1t", [128, KD, DFF], BF16, kind="ExternalInput")
    wgt = nc.dram_tensor("wgt", [128, KD, DFF], BF16, kind="ExternalInput")
    w2t = nc.dram_tensor("w2t", [128, NF, D], BF16, kind="ExternalInput")
    y = nc.dram_tensor("y", [C, D], BF16, kind="ExternalOutput")

    silu = mybir.ActivationFunctionType.Silu

    with tile.TileContext(nc) as tc:
        with (
            tc.tile_pool(name="wres", bufs=1) as wres,
            tc.tile_pool(name="wg", bufs=WG_BUFS) as wgpool,
            tc.tile_pool(name="xt", bufs=XT_BUFS) as xtpool,
            tc.tile_pool(name="hg", bufs=3) as hgpool,
            tc.tile_pool(name="h", bufs=H_BUFS) as hpool,
            tc.tile_pool(name="ht", bufs=NF) as htpool,
            tc.tile_pool(name="w2", bufs=W2_BUFS) as w2pool,
            tc.tile_pool(name="yo", bufs=4) as ypool,
            tc.tile_pool(name="ps1", bufs=1, space="PSUM") as psum1,
            tc.tile_pool(name="ps2", bufs=PSY_BUFS, space="PSUM") as psum2,
        ):
            # Resident w1, split into 8 dff-chunks so the first matmuls only
            # wait on the chunk they need (loaded just-in-time in block 0).
            w1_parts = [
                wres.tile([128, KD, 512], BF16, tag=f"w1p{i}", name=f"w1p{i}")
                for i in range(NF // 4)
            ]

            if repeat > 1:
                # calibration mode: load resident w1 once, outside the loop
                for i in range(NF // 4):
                    nc.sync.dma_start(
                        w1_parts[i][:], w1t[:, :, i * 512:(i + 1) * 512]
                    )

            def _stage1(b, tb, tok0, h_tiles):
                """SwiGLU hidden for tokens [tok0, tok0+tb); appends the 32
                [128, tb] bf16 h tiles to h_tiles."""
                pool = htpool if tb != TB else hpool
                xt_sb = xtpool.tile([128, KD, tb], BF16, tag="xt")
                nc.sync.dma_start(xt_sb[:], xt[:, :, tok0:tok0 + tb])
                for dfc in range(NF // 4):
                    if b == 0 and dfc == 0:
                        # Split the first chunk into 4 tiles so the first
                        # matmul waits on 256 KB, not 1 MB.
                        wg_pieces = [
                            wgpool.tile([128, KD, 128], BF16, bufs=1,
                                        tag=f"wg0p{i}", name=f"wg0p{i}")
                            for i in range(4)
                        ]
                        for i in range(4):
                            nc.sync.dma_start(
                                wg_pieces[i][:],
                                wgt[:, :, i * 128:(i + 1) * 128],
                            )
                        wg_ch = None
                    else:
                        wg_pieces = None
                        wg_ch = wgpool.tile([128, KD, 512], BF16, tag="wg")
                        nc.sync.dma_start(
                            wg_ch[:], wgt[:, :, dfc * 512:(dfc + 1) * 512]
                        )
                    if b == 0 and repeat == 1:
                        nc.sync.dma_start(
                            w1_parts[dfc][:],
                            w1t[:, :, dfc * 512:(dfc + 1) * 512],
                        )
                    for j in range(4):
                        psg = psum1.tile([128, tb], F32, tag="psg",
                                         bufs=PSG_BUFS)
                        for d in range(KD):
                            if wg_pieces is not None:
                                wslice = wg_pieces[j][:, d, :]
                            else:
                                wslice = wg_ch[:, d, j * 128:(j + 1) * 128]
                            nc.tensor.matmul(
                                psg[:],
                                wslice,
                                xt_sb[:, d, :],
                                start=(d == 0),
                                stop=(d == KD - 1),
                            )
                        ps1t = psum1.tile([128, tb], F32, tag="ps1t",
                                          bufs=PS1_BUFS)
                        for d in range(KD):
                            nc.tensor.matmul(
                                ps1t[:],
                                w1_parts[dfc][:, d, j * 128:(j + 1) * 128],
                                xt_sb[:, d, :],
                                start=(d == 0),
                                stop=(d == KD - 1),
                            )
                        hg = hgpool.tile([128, tb], BF16, tag="hg")
                        nc.scalar.activation(hg[:], psg[:], silu)
                        h = pool.tile([128, tb], BF16,
                                      tag="ht" if tb != TB else "h")
                        nc.vector.tensor_mul(h[:], hg[:], ps1t[:])
                        h_tiles.append(h)

            def _stage2(mtiles):
                """One stage-2 pass: mtiles = list of (h_tiles, m, mt, ytok)
                with mt tokens each; every w2 chunk is used by all mtiles."""
                for half in range(2):
                    psys = [
                        psum2.tile([128, 512], F32, tag="psy", name=f"psy{i}")
                        for i in range(len(mtiles))
                    ]
                    for dfp in range(NF // 2):
                        w2_ch = w2pool.tile([128, 2, 512], BF16, tag="w2c")
                        nc.sync.dma_start(
                            w2_ch[:],
                            w2t[:, 2 * dfp:2 * dfp + 2,
                                half * 512:(half + 1) * 512],
                        )
                        for k in range(2):
                            df = 2 * dfp + k
                            for i, (ht, m, mt, _) in enumerate(mtiles):
                                nc.tensor.matmul(
                                    psys[i][:mt, :],
                                    ht[df][:, m * 128:m * 128 + mt],
                                    w2_ch[:, k, :],
                                    start=(df == 0),
                                    stop=(df == NF - 1),
                                )
                    for i, (_, _, mt, ytok) in enumerate(mtiles):
                        y_sb = ypool.tile([128, 512], BF16, tag="ysb")
                        nc.vector.tensor_copy(y_sb[:mt, :], psys[i][:mt, :])
                        nc.sync.dma_start(
                            y[ytok:ytok + mt, half * 512:(half + 1) * 512],
                            y_sb[:mt, :],
                        )

            def _trace_body():
                tok0s = np.concatenate([[0], np.cumsum(blocks)])
                done = 0
                for g in groups:
                    per_block_h = []
                    for b in g:
                        h_tiles = []
                        _stage1(b, blocks[b], int(tok0s[b]), h_tiles)
                        per_block_h.append(h_tiles)
                    mtiles = []
                    for bi, b in enumerate(g):
                        tb, t0 = blocks[b], int(tok0s[b])
                        for m in range((tb + 127) // 128):
                            mt = min(128, tb - m * 128)
                            mtiles.append(
                                (per_block_h[bi], m, mt, t0 + m * 128)
                            )
                    _stage2(mtiles)
                    done += len(g)

            if repeat == 1:
                _trace_body()
            else:
                with tc.For_i(0, repeat, 1):
                    _trace_body()
    _split_multi_waits(nc)
    return nc


def _swizzle_k(a: np.ndarray) -> np.ndarray:
    """[K, F] -> [128, K//128, F] with K = ko*128 + p on partitions."""
    k, f = a.shape
    return np.ascontiguousarray(
        a.reshape(k // 128, 128, f).transpose(1, 0, 2)
    )


def kernel(x, gate_w, w1, w_gate, w2):
    b, t, d = x.shape
    xf = np.ascontiguousarray(x.reshape(-1, d)).astype(np.float32)
    n_tok = xf.shape[0]

    # --- Router (host, fp32, mirrors reference math) ---
    logits = xf @ gate_w.T.astype(np.float32)                  # [N, E]
    top_idx = np.argsort(-logits, axis=1, kind="stable")[:, :TOP_K]  # [N, K]
    top_vals = np.take_along_axis(logits, top_idx, axis=1)
    m = top_vals.max(axis=1, keepdims=True)
    ex = np.exp(top_vals - m)
    top_w = ex / ex.sum(axis=1, keepdims=True)                 # [N, K]

    pair_expert = top_idx.reshape(-1)                          # [N*K]
    pair_w = top_w.reshape(-1).astype(np.float32)
    order = np.argsort(pair_expert, kind="stable")
    counts = np.bincount(pair_expert, minlength=N_EXPERTS)
    starts = np.concatenate([[0], np.cumsum(counts)])

    C = max(128, int(-(-int(counts.max()) // 32)) * 32)

    # --- Build per-core inputs (dispatch) ---
    in_maps = []
    sels = []
    for e in range(N_EXPERTS):
        sel = order[starts[e]:starts[e + 1]]
        sels.append(sel)
        tok = sel // TOP_K
        xt_full = np.zeros((D, C), dtype=np.float32)
        xt_full[:, : len(tok)] = xf[tok].T
        in_maps.append(
            {
                "xt": _swizzle_k(xt_full).astype(NP_BF16),
                "w1t": _swizzle_k(
                    np.ascontiguousarray(w1[e].T).astype(np.float32)
                ).astype(NP_BF16),
                "wgt": _swizzle_k(
                    np.ascontiguousarray(w_gate[e].T).astype(np.float32)
                ).astype(NP_BF16),
                "w2t": _swizzle_k(
                    np.ascontiguousarray(w2[e].T).astype(np.float32)
                ).astype(NP_BF16),
            }
        )

    if C not in _NC_CACHE:
        _NC_CACHE[C] = _build_kernel(C)
    nc = _NC_CACHE[C]

    res = run_bass_kernel_spmd(nc, in_maps, core_ids=list(range(N_CORES)))

    # --- Combine (host): weight by router prob, scatter-add to tokens ---
    contrib = np.zeros((n_tok * TOP_K, D), dtype=np.float32)
    for e in range(N_EXPERTS):
        sel = sels[e]
        y_e = res.results[e]["y"][: len(sel)].astype(np.float32)
        contrib[sel] = y_e * pair_w[sel][:, None]
    out = contrib.reshape(n_tok, TOP_K, D).sum(axis=1)
    return out.reshape(b, t, d).astype(x.dtype)


# revision 7
# speedup vs baseline: 1.0061x; 1.0061x over previous
"""MoE layer (top-2 of 8 experts, SwiGLU) on 8 Trainium2 NeuronCores.

Expert-parallel (per the sharding hint): the host computes the router
(gate logits -> top-2 -> softmax) in fp32, gathers each expert's tokens
(the "all-to-all dispatch"), each core runs a dense SwiGLU MLP over one
expert's tokens in bf16 (fp32 PSUM accumulation), and the host applies
the combine weights and scatter-adds back to token order.

Kernel structure (per core, C = max expert token count padded to 32):
  - stage-1 (h = silu(x@wg.T) * (x@w1.T)) in 512-token blocks plus one
    tail block; w1 resident in SBUF, wg streamed; weights stationary,
    tokens moving (N=512 streams keep LDWEIGHTS hidden).
  - stage-2 (y = h@w2.T) streams w2 two 128x512 chunks per DMA; the
    tail block's stage-2 is merged into the last full block's pass
    (5 psum tiles per w2 chunk) so no pass is w2-DMA-paced and w2 is
    streamed once less per forward.
  - y returned in bf16 (host upcasts; quantization well inside budget).
PSUM: psg 2 + ps1t 1 + psy 5 = 8 banks exactly.
"""

import numpy as np
import ml_dtypes

import concourse.bass as bass
import concourse.mybir as mybir
import concourse.tile as tile
from concourse.bass_utils import run_bass_kernel_spmd

# ---------------------------------------------------------------------------
# Workaround for this walrus build: TPB instructions have a single hardware
# wait slot; split k-wait instructions into k-1 single-wait NOPs + the
# original (program-order semantics identical).
# ---------------------------------------------------------------------------

_ws_counter = [0]


def _split_multi_waits(nc: bass.Bass) -> int:
    n_split = 0
    for f in nc.m.functions:
        for bb in f.blocks:
            new_insts = []
            for inst in bb.instructions:
                si = inst.sync_info
                if si is not None and si.on_wait and len(si.on_wait) > 1:
                    waits = list(si.on_wait)
                    for w in waits[:-1]:
                        _ws_counter[0] += 1
                        n_split += 1
                        new_insts.append(
                            mybir.InstNoOp(
                                name=f"waitsplit-{_ws_counter[0]}",
                                opcode="NoOp",
                                engine=inst.engine,
                                sync_info=mybir.SyncInfo(
                                    on_wait=[w], on_update=[]
                                ),
                                bass_nofuse=True,
                                text_hint="waitsplit",
                            )
                        )
                    si.on_wait = [waits[-1]]
                new_insts.append(inst)
            bb.instructions[:] = new_insts
    return n_split

# ---------------------------------------------------------------------------

D = 1024
DFF = 4096
N_EXPERTS = 8
TOP_K = 2
N_CORES = 8
TB = 512          # full token block
WG_BUFS = 4
XT_BUFS = 3
W2_BUFS = 8       # bufs of 2-chunk w2 tiles
H_BUFS = 36
PSG_BUFS = 2
PS1_BUFS = 1
PSY_BUFS = 5
KD = D // 128     # 8 contraction tiles over d
NF = DFF // 128   # 32 tiles over d_ff

BF16 = mybir.dt.bfloat16
F32 = mybir.dt.float32
NP_BF16 = ml_dtypes.bfloat16

_NC_CACHE: dict[int, bass.Bass] = {}


def _build_kernel(C: int, repeat: int = 1) -> bass.Bass:
    """Dense SwiGLU MLP over C tokens (C a multiple of 32).

    Blocks: full 512-token blocks, then one tail of C%512 (if any). The
    tail's stage-2 is folded into the last full block's stage-2 pass.

    repeat>1 wraps the computation in a hardware For_i loop for
    wall-clock slope calibration (resident w1 loads once, outside)."""
    assert C % 32 == 0 and C >= 128
    tail = C % TB
    n_full = C // TB
    # Tail last: during the earlier blocks' stage-2 passes the wg stream
    # runs ahead, so the tail's fast-consuming stage-1 isn't DMA-paced.
    blocks = [TB] * n_full + ([tail] if tail else [])
    # stage-2 groups: indices of blocks whose stage-2 runs as one pass;
    # the tail's stage-2 is merged with the last full block's when the
    # combined m-tile count fits the PSY_BUFS psum banks.
    groups = [[i] for i in range(len(blocks))]
    if tail and n_full and TB // 128 + (tail + 127) // 128 <= PSY_BUFS:
        groups = groups[:-2] + [[len(blocks) - 2, len(blocks) - 1]]

    nc = bass.Bass()
    xt = nc.dram_tensor("xt", [128, KD, C], BF16, kind="ExternalInput")
    w1t = nc.dram_tensor("w1t", [128, KD, DFF], BF16, kind="ExternalInput")
    wgt = nc.dram_tensor("wgt", [128, KD, DFF], BF16, kind="ExternalInput")
    w2t = nc.dram_tensor("w2t", [128, NF, D], BF16, kind="ExternalInput")
    y = nc.dram_tensor("y", [C, D], BF16, kind="ExternalOutput")

    silu = mybir.ActivationFunctionType.Silu

    with tile.TileContext(nc) as tc:
        with (
            tc.tile_pool(name="wres", bufs=1) as wres,
            tc.tile_pool(name="wg", bufs=WG_BUFS) as wgpool,
            tc.tile_pool(name="xt", bufs=XT_BUFS) as xtpool,
            tc.tile_pool(name="hg", bufs=3) as hgpool,
            tc.tile_pool(name="h", bufs=H_BUFS) as hpool,
            tc.tile_pool(name="ht", bufs=NF) as htpool,
            tc.tile_pool(name="w2", bufs=W2_BUFS) as w2pool,
            tc.tile_pool(name="yo", bufs=4) as ypool,
            tc.tile_pool(name="ps1", bufs=1, space="PSUM") as psum1,
            tc.tile_pool(name="ps2", bufs=PSY_BUFS, space="PSUM") as psum2,
        ):
            # Resident w1, split into 8 dff-chunks so the first matmuls only
            # wait on the chunk they need (loaded just-in-time in block 0).
            w1_parts = [
                wres.tile([128, KD, 512], BF16, tag=f"w1p{i}", name=f"w1p{i}")
                for i in range(NF // 4)
            ]

            if repeat > 1:
                # calibration mode: load resident w1 once, outside the loop
                for i in range(NF // 4):
                    nc.sync.dma_start(
                        w1_parts[i][:], w1t[:, :, i * 512:(i + 1) * 512]
                    )

            def _stage1(b, tb, tok0, h_tiles):
                """SwiGLU hidden for tokens [tok0, tok0+tb); appends the 32
                [128, tb] bf16 h tiles to h_tiles."""
                pool = htpool if tb != TB else hpool
                xt_sb = xtpool.tile([128, KD, tb], BF16, tag="xt")
                nc.sync.dma_start(xt_sb[:], xt[:, :, tok0:tok0 + tb])
                for dfc in range(NF // 4):
                    if b == 0 and dfc == 0:
                        # Split the first chunk into 4 tiles so the first
                        # matmul waits on 256 KB, not 1 MB.
                        wg_pieces = [
                            wgpool.tile([128, KD, 128], BF16, bufs=1,
                                        tag=f"wg0p{i}", name=f"wg0p{i}")
                            for i in range(4)
                        ]
                        for i in range(4):
                            nc.sync.dma_start(
                                wg_pieces[i][:],
                                wgt[:, :, i * 128:(i + 1) * 128],
                            )
                        wg_ch = None
                    else:
                        wg_pieces = None
                        wg_ch = wgpool.tile([128, KD, 512], BF16, tag="wg")
                        nc.sync.dma_start(
                            wg_ch[:], wgt[:, :, dfc * 512:(dfc + 1) * 512]
                        )
                    if b == 0 and repeat == 1:
                        nc.sync.dma_start(
                            w1_parts[dfc][:],
                            w1t[:, :, dfc * 512:(dfc + 1) * 512],
                        )
                    for j in range(4):
                        psg = psum1.tile([128, tb], F32, tag="psg",
                                         bufs=PSG_BUFS)
                        for d in range(KD):
                            if wg_pieces is not None:
                                wslice = wg_pieces[j][:, d, :]
                            else:
                                wslice = wg_ch[:, d, j * 128:(j + 1) * 128]
                            nc.tensor.matmul(
                                psg[:],
                                wslice,
                                xt_sb[:, d, :],
                                start=(d == 0),
                                stop=(d == KD - 1),
                            )
                        ps1t = psum1.tile([128, tb], F32, tag="ps1t",
                                          bufs=PS1_BUFS)
                        for d in range(KD):
                            nc.tensor.matmul(
                                ps1t[:],
                                w1_parts[dfc][:, d, j * 128:(j + 1) * 128],
                                xt_sb[:, d, :],
                                start=(d == 0),
                                stop=(d == KD - 1),
                            )
                        hg = hgpool.tile([128, tb], BF16, tag="hg")
                        nc.scalar.activation(hg[:], psg[:], silu)
                        h = pool.tile([128, tb], BF16,
                                      tag="ht" if tb != TB else "h")
                        nc.vector.tensor_mul(h[:], hg[:], ps1t[:])
                        h_tiles.append(h)

            def _stage2(mtiles):
                """One stage-2 pass: mtiles = list of (h_tiles, m, mt, ytok)
                with mt tokens each; every w2 chunk is used by all mtiles."""
                for half in range(2):
                    psys = [
                        psum2.tile([128, 512], F32, tag="psy", name=f"psy{i}")
                        for i in range(len(mtiles))
                    ]
                    for dfp in range(NF // 2):
                        w2_ch = w2pool.tile([128, 2, 512], BF16, tag="w2c")
                        nc.sync.dma_start(
                            w2_ch[:],
                            w2t[:, 2 * dfp:2 * dfp + 2,
                                half * 512:(half + 1) * 512],
                        )
                        for k in range(2):
                            df = 2 * dfp + k
                            for i, (ht, m, mt, _) in enumerate(mtiles):
                                nc.tensor.matmul(
                                    psys[i][:mt, :],
                                    ht[df][:, m * 128:m * 128 + mt],
                                    w2_ch[:, k, :],
                                    start=(df == 0),
                                    stop=(df == NF - 1),
                                )
                    for i, (_, _, mt, ytok) in enumerate(mtiles):
                        y_sb = ypool.tile([128, 512], BF16, tag="ysb")
                        nc.vector.tensor_copy(y_sb[:mt, :], psys[i][:mt, :])
                        nc.sync.dma_start(
                            y[ytok:ytok + mt, half * 512:(half + 1) * 512],
                            y_sb[:mt, :],
                        )

            def _trace_body():
                tok0s = np.concatenate([[0], np.cumsum(blocks)])
                done = 0
                for g in groups:
                    per_block_h = []
                    for b in g:
                        h_tiles = []
                        _stage1(b, blocks[b], int(tok0s[b]), h_tiles)
                        per_block_h.append(h_tiles)
                    mtiles = []
                    for bi, b in enumerate(g):
                        tb, t0 = blocks[b], int(tok0s[b])
                        for m in range((tb + 127) // 128):
                            mt = min(128, tb - m * 128)
                            mtiles.append(
                                (per_block_h[bi], m, mt, t0 + m * 128)
                            )
                    _stage2(mtiles)
                    done += len(g)

            if repeat == 1:
                _trace_body()
            else:
                with tc.For_i(0, repeat, 1):
                    _trace_body()
    _split_multi_waits(nc)
    return nc


def _swizzle_k(a: np.ndarray) -> np.ndarray:
    """[K, F] -> [128, K//128, F] with K = ko*128 + p on partitions."""
    k, f = a.shape
    return np.ascontiguousarray(
        a.reshape(k // 128, 128, f).transpose(1, 0, 2)
    )


def kernel(x, gate_w, w1, w_gate, w2):
    b, t, d = x.shape
    xf = np.ascontiguousarray(x.reshape(-1, d)).astype(np.float32)
    n_tok = xf.shape[0]

    # --- Router (host, fp32, mirrors reference math) ---
    logits = xf @ gate_w.T.astype(np.float32)                  # [N, E]
    top_idx = np.argsort(-logits, axis=1, kind="stable")[:, :TOP_K]  # [N, K]
    top_vals = np.take_along_axis(logits, top_idx, axis=1)
    m = top_vals.max(axis=1, keepdims=True)
    ex = np.exp(top_vals - m)
    top_w = ex / ex.sum(axis=1, keepdims=True)                 # [N, K]

    pair_expert = top_idx.reshape(-1)                          # [N*K]
    pair_w = top_w.reshape(-1).astype(np.float32)
    order = np.argsort(pair_expert, kind="stable")
    counts = np.bincount(pair_expert, minlength=N_EXPERTS)
    starts = np.concatenate([[0], np.cumsum(counts)])

    C = max(128, int(-(-int(counts.max()) // 32)) * 32)

    # --- Build per-core inputs (dispatch) ---
    in_maps = []
    sels = []
    for e in range(N_EXPERTS):
        sel = order[starts[e]:starts[e + 1]]
        sels.append(sel)
        tok = sel // TOP_K
        xt_full = np.zeros((D, C), dtype=np.float32)
        xt_full[:, : len(tok)] = xf[tok].T
        in_maps.append(
            {
                "xt": _swizzle_k(xt_full).astype(NP_BF16),
                "w1t": _swizzle_k(
                    np.ascontiguousarray(w1[e].T).astype(np.float32)
                ).astype(NP_BF16),
                "wgt": _swizzle_k(
                    np.ascontiguousarray(w_gate[e].T).astype(np.float32)
                ).astype(NP_BF16),
                "w2t": _swizzle_k(
                    np.ascontiguousarray(w2[e].T).astype(np.float32)
                ).astype(NP_BF16),
            }
        )

    if C not in _NC_CACHE:
        _NC_CACHE[C] = _build_kernel(C)
    nc = _NC_CACHE[C]

    res = run_bass_kernel_spmd(nc, in_maps, core_ids=list(range(N_CORES)))

    # --- Combine (host): weight by router prob, scatter-add to tokens ---
    contrib = np.zeros((n_tok * TOP_K, D), dtype=np.float32)
    for e in range(N_EXPERTS):
        sel = sels[e]
        y_e = res.results[e]["y"][: len(sel)].astype(np.float32)
        contrib[sel] = y_e * pair_w[sel][:, None]
    out = contrib.reshape(n_tok, TOP_K, D).sum(axis=1)
    return out.reshape(b, t, d).astype(x.dtype)
